# revision 11
# baseline (speedup 1.0000x reference)
"""Single-head causal attention with softmax over the QUERY axis (dim=1).

out[b,i,d] = sum_j softmax_i(mask(q@kT/8))[i,j] * v[j,d]

Strategy: data-parallel over batch B=8, one batch element per NeuronCore.
Per core:
  - transpose x[b] (PE transposes) -> xT [C=384, S=2048]
  - qT = (Wq/8).T @ xT, kT = Wk.T @ xT   (both [64, 2048], d on partitions)
  - v  = x @ Wv                          ([2048, 64] as 16 [128,64] tiles)
  - for each key tile jt: scoresT[j, i] = kT_jt.T @ qT  (j on partitions,
    i on free axis) => softmax over i is a FREE-AXIS reduction, fused into
    the Exp activation via accum_out.  Causal mask handled additively on
    the diagonal block only (i >= j valid).
  - fold 1/denom_j into v rows: vs[j,:] = v[j,:] / denom[j], then
    out[i,:] += attnT_jt[:, i].T @ vs  accumulated in PSUM across jt.
"""

import numpy as np
import sys

sys.path.insert(0, "/opt/trn_rl_repo")

import concourse.bass as bass
import concourse.mybir as mybir
from concourse.bacc import Bacc
from concourse.tile import TileContext
from concourse.bass_utils import run_bass_kernel_spmd

B, S, C, D = 8, 2048, 384, 64
P = 128
NT = S // P  # 16 query/key tiles
CC = C // P  # 3 contraction chunks
F32 = mybir.dt.float32
F32R = mybir.dt.float32r
AFT = mybir.ActivationFunctionType
AX = mybir.AxisListType

_COMPILED = None
BUFS = {"ps": 6, "attnp": 3, "small": 6, "xsp": 4}


def build_nc():
    nc = Bacc()
    x_b = nc.declare_dram_parameter("x_b", [S, C], F32R, isOutput=False)
    wq = nc.declare_dram_parameter("wq", [C, D], F32R, isOutput=False)  # pre-scaled 1/8
    wk = nc.declare_dram_parameter("wk", [C, D], F32R, isOutput=False)
    wv = nc.declare_dram_parameter("wv", [C, D], F32, isOutput=False)
    ident = nc.declare_dram_parameter("ident", [P, P], F32R, isOutput=False)
    negmask = nc.declare_dram_parameter("negmask", [P, P], F32, isOutput=False)
    out_b = nc.declare_dram_parameter("out_b", [S, D], F32, isOutput=True)

    with TileContext(nc) as tc:
        with (
            tc.tile_pool(name="consts", bufs=1) as consts,
            tc.tile_pool(name="big", bufs=1) as big,
            tc.tile_pool(name="xsp", bufs=BUFS["xsp"]) as xsp,
            tc.tile_pool(name="attnp", bufs=BUFS["attnp"]) as attnp,
            tc.tile_pool(name="small", bufs=BUFS["small"]) as small,
            tc.tile_pool(name="psO", bufs=1, space="PSUM") as psO,
            tc.tile_pool(name="ps", bufs=BUFS["ps"], space="PSUM") as ps,
        ):
            # ---- constants ----
            idt = consts.tile([P, P], F32R)
            nc.sync.dma_start(out=idt, in_=ident[:, :])
            msk = consts.tile([P, P], F32)
            nc.sync.dma_start(out=msk, in_=negmask[:, :])
            wq_t = consts.tile([P, CC * D], F32R)
            wk_t = consts.tile([P, CC * D], F32R)
            wv_t = consts.tile([P, CC * D], F32)
            for wt, wd in ((wq_t, wq), (wk_t, wk), (wv_t, wv)):
                nc.sync.dma_start(
                    out=wt.rearrange("p (c d) -> p c d", c=CC),
                    in_=wd.ap().rearrange("(c p) d -> p c d", p=P),
                )

            # ---- persistent SBUF tensors ----
            xT = big.tile([P, CC * S], F32R)        # [128, 3*2048] xT chunks
            qk = big.tile([64, 2 * S], F32R)        # qT(scaled) | kT
            v_all = big.tile([P, NT * D], F32)     # v tiles [128, 16*64]
            out_sb = big.tile([P, NT * D], F32)    # final out staging

            # ---- phase A: load + transpose x (4 s-tiles per DMA) ----
            for g in range(NT // 4):
                xs = xsp.tile([P, 4 * C], F32R, tag="xs")
                nc.sync.dma_start(
                    out=xs.rearrange("p (t c) -> p t c", t=4),
                    in_=x_b[g * 4 * P:(g + 1) * 4 * P, :].rearrange(
                        "(t p) c -> p t c", p=P),
                )
                for c in range(CC):
                    pt4 = ps.tile([P, 512], F32, tag="ps")
                    for t in range(4):
                        nc.tensor.matmul(
                            pt4[:, t * P:(t + 1) * P].bitcast(F32R),
                            xs[:, t * C + c * P: t * C + (c + 1) * P], idt,
                            is_transpose=True, start=(t == 0), stop=(t == 3),
                        )
                    nc.vector.tensor_copy(
                        xT[:, c * S + g * 4 * P: c * S + (g + 1) * 4 * P], pt4
                    )

            # ---- qT / kT: [64, 2048] = W.T @ xT ----
            for n in range(S // 512):
                pq = ps.tile([64, 512], F32, tag="ps")
                for c in range(CC):
                    nc.tensor.matmul(
                        pq, wq_t[:, c * D:(c + 1) * D],
                        xT[:, c * S + n * 512: c * S + (n + 1) * 512],
                        start=(c == 0), stop=(c == CC - 1),
                    )
                nc.vector.tensor_copy(qk[:, n * 512:(n + 1) * 512], pq)
                pk = ps.tile([64, 512], F32, tag="ps")
                for c in range(CC):
                    nc.tensor.matmul(
                        pk, wk_t[:, c * D:(c + 1) * D],
                        xT[:, c * S + n * 512: c * S + (n + 1) * 512],
                        start=(c == 0), stop=(c == CC - 1),
                    )
                nc.vector.tensor_copy(qk[:, S + n * 512: S + (n + 1) * 512], pk)

            # ---- v tiles [128, 64] = xT_chunk.T @ Wv ----
            for st in range(NT):
                pv = ps.tile([P, D], F32, tag="ps")
                for c in range(CC):
                    nc.tensor.matmul(
                        pv, xT[:, c * S + st * P: c * S + (st + 1) * P].bitcast(F32),
                        wv_t[:, c * D:(c + 1) * D],
                        start=(c == 0), stop=(c == CC - 1),
                    )
                nc.vector.tensor_copy(v_all[:, st * D:(st + 1) * D], pv)

            # ---- phase B: per key-tile softmax + accumulation ----
            # Software-pipelined: scores+exp for jt+1 are emitted BEFORE the
            # softmax tail + attn@v matmuls of jt, so PE works on scores_{jt+1}
            # while ACT/DVE finish the softmax chain of jt.
            outp = psO.tile([P, NT * D], F32)  # [128, 1024] accumulator, 2 banks

            def emit_scores(jt):
                Ni = S - jt * P  # valid queries i >= jt*128
                atile = attnp.tile([P, S], F32, tag="attn", name=f"atile{jt}")
                dens = small.tile([P, 4], F32, tag="dens", name=f"dens{jt}")
                nch = (Ni + 511) // 512
                for ci in range(nch):
                    w = min(512, Ni - ci * 512)
                    i0 = jt * P + ci * 512
                    sc = ps.tile([P, 512], F32, tag="ps", name=f"sc{jt}_{ci}")
                    nc.tensor.matmul(
                        sc[:, :w],
                        qk[:, S + jt * P: S + (jt + 1) * P],
                        qk[:, i0: i0 + w],
                        start=True, stop=True,
                    )
                    if ci == 0:
                        # causal mask on diagonal block: -1e30 where i < j
                        nc.vector.tensor_add(sc[:, :P], sc[:, :P], msk)
                    nc.scalar.activation(
                        atile[:, ci * 512: ci * 512 + w], sc[:, :w], AFT.Exp,
                        accum_out=dens[:, ci: ci + 1],
                    )
                return atile, dens, nch

            pend = emit_scores(0)
            for jt in range(NT):
                atile, dens, nch = pend
                if jt + 1 < NT:
                    pend = emit_scores(jt + 1)
                den = small.tile([P, 1], F32, tag="den")
                nc.vector.reduce_sum(den, dens[:, :nch], axis=AX.X)
                rv = small.tile([P, 1], F32, tag="rv")
                nc.vector.reciprocal(rv, den)
                vs = small.tile([P, D], F32, tag="vs")
                nc.vector.tensor_scalar_mul(vs, v_all[:, jt * D:(jt + 1) * D], rv)
                for it in range(jt, NT):
                    # outp is 2 PSUM banks (it 0..7 | 8..15). start=True zeroes
                    # the whole 2KB bank, so only the first matmul touching each
                    # bank starts; the last touching each bank stops.
                    bank_first = jt == 0 and it in (0, 8)
                    bank_last = (jt == 7 and it == 7) or (jt == 15 and it == 15)
                    nc.tensor.matmul(
                        outp[:, it * D:(it + 1) * D],
                        atile[:, (it - jt) * P:(it - jt + 1) * P],  # [128j,128i]
                        vs,
                        start=bank_first, stop=bank_last,
                    )

            nc.vector.tensor_copy(out_sb, outp)
            nc.sync.dma_start(
                out=out_b.ap().rearrange("(t p) d -> p t d", p=P),
                in_=out_sb.rearrange("p (t d) -> p t d", t=NT),
            )
    nc.finalize()
    return nc


def _build_inputs(x, Wq, Wk, Wv):
    x = np.ascontiguousarray(np.asarray(x, dtype=np.float32))
    wq_s = np.ascontiguousarray(np.asarray(Wq, dtype=np.float32) * np.float32(D ** -0.5))
    wk_ = np.ascontiguousarray(np.asarray(Wk, dtype=np.float32))
    wv_ = np.ascontiguousarray(np.asarray(Wv, dtype=np.float32))
    ident = np.eye(P, dtype=np.float32)
    r = np.arange(P)
    negmask = np.where(r[None, :] >= r[:, None], 0.0, -1e30).astype(np.float32)
    return [
        {"x_b": x[b], "wq": wq_s, "wk": wk_, "wv": wv_,
         "ident": ident, "negmask": negmask}
        for b in range(B)
    ]


def kernel(x, Wq, Wk, Wv, _trace=False):
    global _COMPILED
    if _COMPILED is None:
        _COMPILED = build_nc()
    nc = _COMPILED
    in_maps = _build_inputs(x, Wq, Wk, Wv)
    res = run_bass_kernel_spmd(nc, in_maps, core_ids=list(range(B)), trace=_trace)
    out = np.stack([res.results[b]["out_b"] for b in range(B)], axis=0).astype(np.float32)
    if _trace:
        kernel.last_results = res
    return out


# revision 12
# speedup vs baseline: 1.0005x; 1.0005x over previous
"""Single-head causal attention with softmax over the QUERY axis (dim=1).

out[b,i,d] = sum_j softmax_i(mask(q@kT/8))[i,j] * v[j,d]

Strategy: data-parallel over batch B=8, one batch element per NeuronCore.
Per core:
  - transpose x[b] (PE transposes) -> xT [C=384, S=2048]
  - qT = (Wq/8).T @ xT, kT = Wk.T @ xT   (both [64, 2048], d on partitions)
  - v  = x @ Wv                          ([2048, 64] as 16 [128,64] tiles)
  - for each key tile jt: scoresT[j, i] = kT_jt.T @ qT  (j on partitions,
    i on free axis) => softmax over i is a FREE-AXIS reduction, fused into
    the Exp activation via accum_out.  Causal mask handled additively on
    the diagonal block only (i >= j valid).
  - fold 1/denom_j into v rows: vs[j,:] = v[j,:] / denom[j], then
    out[i,:] += attnT_jt[:, i].T @ vs  accumulated in PSUM across jt.
"""

import numpy as np
import sys

sys.path.insert(0, "/opt/trn_rl_repo")

import concourse.bass as bass
import concourse.mybir as mybir
from concourse.bacc import Bacc
from concourse.tile import TileContext
from concourse.bass_utils import run_bass_kernel_spmd

B, S, C, D = 8, 2048, 384, 64
P = 128
NT = S // P  # 16 query/key tiles
CC = C // P  # 3 contraction chunks
F32 = mybir.dt.float32
F32R = mybir.dt.float32r
AFT = mybir.ActivationFunctionType
AX = mybir.AxisListType

_COMPILED = None
BUFS = {"ps": 6, "attnp": 3, "small": 6, "xsp": 4}


def build_nc():
    nc = Bacc()
    x_b = nc.declare_dram_parameter("x_b", [S, C], F32R, isOutput=False)
    wq = nc.declare_dram_parameter("wq", [C, D], F32R, isOutput=False)  # pre-scaled 1/8
    wk = nc.declare_dram_parameter("wk", [C, D], F32R, isOutput=False)
    wv = nc.declare_dram_parameter("wv", [C, D], F32, isOutput=False)
    ident = nc.declare_dram_parameter("ident", [P, P], F32R, isOutput=False)
    negmask = nc.declare_dram_parameter("negmask", [P, P], F32, isOutput=False)
    out_b = nc.declare_dram_parameter("out_b", [S, D], F32, isOutput=True)

    with TileContext(nc) as tc:
        with (
            tc.tile_pool(name="consts", bufs=1) as consts,
            tc.tile_pool(name="big", bufs=1) as big,
            tc.tile_pool(name="xsp", bufs=BUFS["xsp"]) as xsp,
            tc.tile_pool(name="attnp", bufs=BUFS["attnp"]) as attnp,
            tc.tile_pool(name="small", bufs=BUFS["small"]) as small,
            tc.tile_pool(name="psO", bufs=1, space="PSUM") as psO,
            tc.tile_pool(name="ps", bufs=BUFS["ps"], space="PSUM") as ps,
        ):
            # ---- constants ----
            idt = consts.tile([P, P], F32R)
            nc.sync.dma_start(out=idt, in_=ident[:, :])
            msk = consts.tile([P, P], F32)
            nc.sync.dma_start(out=msk, in_=negmask[:, :])
            wq_t = consts.tile([P, CC * D], F32R)
            wk_t = consts.tile([P, CC * D], F32R)
            wv_t = consts.tile([P, CC * D], F32)
            for wt, wd in ((wq_t, wq), (wk_t, wk), (wv_t, wv)):
                nc.sync.dma_start(
                    out=wt.rearrange("p (c d) -> p c d", c=CC),
                    in_=wd.ap().rearrange("(c p) d -> p c d", p=P),
                )

            # ---- persistent SBUF tensors ----
            xT = big.tile([P, CC * S], F32R)        # [128, 3*2048] xT chunks
            qk = big.tile([64, 2 * S], F32R)        # qT(scaled) | kT
            v_all = big.tile([P, NT * D], F32)     # v tiles [128, 16*64]
            out_sb = big.tile([P, NT * D], F32)    # final out staging

            # ---- phase A: load + transpose x (4 s-tiles per DMA) ----
            for g in range(NT // 4):
                xs = xsp.tile([P, 4 * C], F32R, tag="xs")
                nc.sync.dma_start(
                    out=xs.rearrange("p (t c) -> p t c", t=4),
                    in_=x_b[g * 4 * P:(g + 1) * 4 * P, :].rearrange(
                        "(t p) c -> p t c", p=P),
                )
                for c in range(CC):
                    pt4 = ps.tile([P, 512], F32, tag="ps")
                    for t in range(4):
                        nc.tensor.matmul(
                            pt4[:, t * P:(t + 1) * P].bitcast(F32R),
                            xs[:, t * C + c * P: t * C + (c + 1) * P], idt,
                            is_transpose=True, start=(t == 0), stop=(t == 3),
                        )
                    nc.vector.tensor_copy(
                        xT[:, c * S + g * 4 * P: c * S + (g + 1) * 4 * P], pt4
                    )

            # ---- qT / kT: [64, 2048] = W.T @ xT ----
            for n in range(S // 512):
                pq = ps.tile([64, 512], F32, tag="ps")
                for c in range(CC):
                    nc.tensor.matmul(
                        pq, wq_t[:, c * D:(c + 1) * D],
                        xT[:, c * S + n * 512: c * S + (n + 1) * 512],
                        start=(c == 0), stop=(c == CC - 1),
                    )
                nc.vector.tensor_copy(qk[:, n * 512:(n + 1) * 512], pq)
                pk = ps.tile([64, 512], F32, tag="ps")
                for c in range(CC):
                    nc.tensor.matmul(
                        pk, wk_t[:, c * D:(c + 1) * D],
                        xT[:, c * S + n * 512: c * S + (n + 1) * 512],
                        start=(c == 0), stop=(c == CC - 1),
                    )
                nc.vector.tensor_copy(qk[:, S + n * 512: S + (n + 1) * 512], pk)

            # ---- v tiles [128, 64] = xT_chunk.T @ Wv ----
            for st in range(NT):
                pv = ps.tile([P, D], F32, tag="ps")
                for c in range(CC):
                    nc.tensor.matmul(
                        pv, xT[:, c * S + st * P: c * S + (st + 1) * P].bitcast(F32),
                        wv_t[:, c * D:(c + 1) * D],
                        start=(c == 0), stop=(c == CC - 1),
                    )
                nc.vector.tensor_copy(v_all[:, st * D:(st + 1) * D], pv)

            # ---- phase B: per key-tile softmax + accumulation ----
            # Software-pipelined: scores+exp for jt+1 are emitted BEFORE the
            # softmax tail + attn@v matmuls of jt, so PE works on scores_{jt+1}
            # while ACT/DVE finish the softmax chain of jt.
            outp = psO.tile([P, NT * D], F32)  # [128, 1024] accumulator, 2 banks

            def emit_scores(jt):
                Ni = S - jt * P  # valid queries i >= jt*128
                atile = attnp.tile([P, S], F32, tag="attn", name=f"atile{jt}")
                dens = small.tile([P, 4], F32, tag="dens", name=f"dens{jt}")
                nch = (Ni + 511) // 512
                for ci in range(nch):
                    w = min(512, Ni - ci * 512)
                    i0 = jt * P + ci * 512
                    sc = ps.tile([P, 512], F32, tag="ps", name=f"sc{jt}_{ci}")
                    nc.tensor.matmul(
                        sc[:, :w],
                        qk[:, S + jt * P: S + (jt + 1) * P],
                        qk[:, i0: i0 + w],
                        start=True, stop=True,
                    )
                    if ci == 0:
                        # causal mask on diagonal block: -1e30 where i < j
                        nc.vector.tensor_add(sc[:, :P], sc[:, :P], msk)
                    nc.scalar.activation(
                        atile[:, ci * 512: ci * 512 + w], sc[:, :w], AFT.Exp,
                        accum_out=dens[:, ci: ci + 1],
                    )
                return atile, dens, nch

            pend = emit_scores(0)
            for jt in range(NT):
                atile, dens, nch = pend
                if jt + 1 < NT:
                    pend = emit_scores(jt + 1)
                if nch == 1:
                    den = dens[:, 0:1]  # single chunk: accum_out IS the row sum
                else:
                    den_t = small.tile([P, 1], F32, tag="den")
                    nc.vector.reduce_sum(den_t, dens[:, :nch], axis=AX.X)
                    den = den_t
                rv = small.tile([P, 1], F32, tag="rv")
                nc.vector.reciprocal(rv, den)
                vs = small.tile([P, D], F32, tag="vs")
                nc.vector.tensor_scalar_mul(vs, v_all[:, jt * D:(jt + 1) * D], rv)
                for it in range(jt, NT):
                    # outp is 2 PSUM banks (it 0..7 | 8..15). start=True zeroes
                    # the whole 2KB bank, so only the first matmul touching each
                    # bank starts; the last touching each bank stops.
                    bank_first = jt == 0 and it in (0, 8)
                    bank_last = (jt == 7 and it == 7) or (jt == 15 and it == 15)
                    nc.tensor.matmul(
                        outp[:, it * D:(it + 1) * D],
                        atile[:, (it - jt) * P:(it - jt + 1) * P],  # [128j,128i]
                        vs,
                        start=bank_first, stop=bank_last,
                    )

            nc.vector.tensor_copy(out_sb, outp)
            nc.sync.dma_start(
                out=out_b.ap().rearrange("(t p) d -> p t d", p=P),
                in_=out_sb.rearrange("p (t d) -> p t d", t=NT),
            )
    nc.finalize()
    return nc


def _build_inputs(x, Wq, Wk, Wv):
    x = np.ascontiguousarray(np.asarray(x, dtype=np.float32))
    wq_s = np.ascontiguousarray(np.asarray(Wq, dtype=np.float32) * np.float32(D ** -0.5))
    wk_ = np.ascontiguousarray(np.asarray(Wk, dtype=np.float32))
    wv_ = np.ascontiguousarray(np.asarray(Wv, dtype=np.float32))
    ident = np.eye(P, dtype=np.float32)
    r = np.arange(P)
    negmask = np.where(r[None, :] >= r[:, None], 0.0, -1e30).astype(np.float32)
    return [
        {"x_b": x[b], "wq": wq_s, "wk": wk_, "wv": wv_,
         "ident": ident, "negmask": negmask}
        for b in range(B)
    ]


def kernel(x, Wq, Wk, Wv, _trace=False):
    global _COMPILED
    if _COMPILED is None:
        _COMPILED = build_nc()
    nc = _COMPILED
    in_maps = _build_inputs(x, Wq, Wk, Wv)
    res = run_bass_kernel_spmd(nc, in_maps, core_ids=list(range(B)), trace=_trace)
    out = np.stack([res.results[b]["out_b"] for b in range(B)], axis=0).astype(np.float32)
    if _trace:
        kernel.last_results = res
    return out
